# revision 16
# baseline (speedup 1.0000x reference)
"""Trainium2 Bass kernel for nn_Decoder: bit-unpack 23x22-bit codes per batch
row, gather fp16 table rows by index, sign-flip about 0.5, scatter into a
[B, 2, 126, 128] fp32 output whose rows 19:67 carry data and the rest are 0.5.

Sharding: data-parallel over batch across 8 NeuronCores (1024 rows each); the
lookup table is replicated on every core.

Table repack (host-side, untimed): the original row is [2, 48, 8] fp16 =
1536B, but codes 0..13 only consume a 4-channel half ([2,48,0:4] for c<7,
[2,48,4:8] for 7<=c<14). We upload TN2[L, 768] fp16 whose row i is
[lo-half(i) | hi-half(i)]; narrow codes gather 768B at element_offset 0/384,
wide codes (14..22) gather the full 1536B row. Cuts gather HBM reads from
35328B to 24576B per batch row with a single 201MB table.

HW indirect gather consumes ONE offset per partition and fetches a contiguous
per-partition block (probe-verified; CoreSim's multi-offset generality does
NOT hold on HW) -> one DMA per code, 23 per group; the ~1.1us/instruction
Pool desc-gen makes the gather stream span ~330us regardless of scheduling.

Ring schedule (best of 7 traced variants): ACT ring carries a deep backlog
of constant fills so it never idles; SP ring is dedicated to the whole
per-group stores (a store queued behind fills inherits their FIFO drain
delay straight onto the od-recycle critical path; splitting stores across
rings or pushing them through SWDGE both measured slower). The last few
fills are deferred: emitted on BOTH rings after the final store so the
write tail drains dual-ring instead of ACT-alone. Gathers ride SWDGE q0/q1
with 2 groups of buffering to keep the Pool desc-gen stream smooth.

Self-contained: hardcodes all shapes; no imports from the problem directory.
"""

import numpy as np

import concourse.bacc as bacc
import concourse.bass as bass
import concourse.mybir as mybir
import concourse.tile as tile

# Problem constants (hardcoded per contract)
BATCH = 8192
XCOLS = 512          # 6 + 23*22
NCODE = 23
NBITS = 22
L = 131072           # table rows
ROW = 768            # fp16 elements per repacked row [lo 384 | hi 384]
HROW = 384
NCORES = 8
BC = BATCH // NCORES  # 1024 rows per core
P = 128
GROUPS = BC // P      # 8 groups of 128 batch rows

# Output geometry: out[b] is [2, 126, 128] fp32 = [p, r, c].
# Data rows are r in [19, 67); flattened per-b layout [32256]:
#   [0:2432) = 0.5 | [2432:8576) p0 data | [8576:18560) = 0.5 |
#   [18560:24704) p1 data | [24704:32256) = 0.5
F_ROW = 126 * 128     # 16128 per p
D_LO = 19 * 128       # 2432
D_HI = 67 * 128       # 8576
GAP_MID = (126 - 67 + 19) * 128   # 9984
GAP_HI = (126 - 67) * 128         # 7552

# Fill spans per group (each <= GAP_HI wide, the c05 source width).
FILL_SPANS = (
    (0, D_LO),
    (D_HI, D_HI + GAP_HI),
    (D_HI + GAP_HI, D_HI + GAP_MID),
    (F_ROW + D_HI, 2 * F_ROW),
)
N_TAIL_FILLS = 0      # deferred to both rings after the last store

f16 = mybir.dt.float16
f32 = mybir.dt.float32
i32 = mybir.dt.int32


N_SWDGE_QUEUES = 2


def build_module():
    nc = bacc.Bacc(
        "TRN2", target_bir_lowering=False, debug=False,
        num_swdge_queues=N_SWDGE_QUEUES,
    )
    x_t = nc.dram_tensor("x", [BC, XCOLS], i32, kind="ExternalInput")
    tn_t = nc.dram_tensor("table", [L, ROW], f16, kind="ExternalInput")
    w_t = nc.dram_tensor("w", [P, NCODE * NBITS], f32, kind="ExternalInput")
    out_t = nc.dram_tensor("out", [BC, 2, 126, 128], f32, kind="ExternalOutput")

    outf = out_t[:].rearrange("b p r c -> b (p r c)")    # [BC, 32256]
    out3 = out_t[:].rearrange("b p r c -> b p (r c)")    # [BC, 2, 16128]

    regions = [(lo, hi, g) for g in range(GROUPS) for lo, hi in FILL_SPANS]

    with tile.TileContext(nc) as tc:
        with (
            tc.tile_pool(name="const", bufs=1) as cpool,
            tc.tile_pool(name="xp", bufs=2) as xpool,
            tc.tile_pool(name="sm", bufs=GROUPS) as spool,
            tc.tile_pool(name="gn", bufs=28) as gnpool,
            tc.tile_pool(name="gw", bufs=20) as gwpool,
            tc.tile_pool(name="op", bufs=2) as opool,
        ):
            w_tile = cpool.tile([P, NCODE * NBITS], f32)
            nc.sync.dma_start(w_tile[:], w_t[:])
            c05 = cpool.tile([P, GAP_HI], f32)
            nc.vector.memset(c05[:], 0.5)

            def fill(eng, rs):
                for lo, hi, g in rs:
                    b0 = g * P
                    eng.dma_start(
                        out=outf[b0 : b0 + P, lo:hi], in_=c05[:, 0 : hi - lo]
                    )

            # Deep fill backlog on the ACT ring: it never idles mid-run.
            fill(
                nc.scalar,
                regions[:-N_TAIL_FILLS] if N_TAIL_FILLS else regions,
            )

            # Decode all idx/sign tiles up-front so the gather stream is
            # never gated on the Vector chain mid-flight. x loads ride the
            # SP ring ahead of every store.
            idxs, tts, sgs = [], [], []
            for g in range(GROUPS):
                b0 = g * P
                x_tile = xpool.tile([P, XCOLS], i32)
                nc.sync.dma_start(x_tile[:], x_t[b0 : b0 + P, :])
                xf = xpool.tile([P, XCOLS], f32)
                nc.vector.tensor_copy(out=xf[:], in_=x_tile[:])
                prod = xpool.tile([P, NCODE * NBITS], f32)
                nc.vector.tensor_tensor(
                    out=prod[:], in0=xf[:, 6:], in1=w_tile[:],
                    op=mybir.AluOpType.mult,
                )
                codes = xpool.tile([P, NCODE], f32, tag="codes")
                nc.vector.tensor_reduce(
                    out=codes[:],
                    in_=prod[:].rearrange("n (c a) -> n c a", a=NBITS),
                    axis=mybir.AxisListType.X,
                    op=mybir.AluOpType.add,
                )
                codesi = xpool.tile([P, NCODE], i32, tag="codesi")
                nc.vector.tensor_copy(out=codesi[:], in_=codes[:])
                idx = spool.tile([P, NCODE], i32, tag="idx")
                nc.vector.tensor_scalar(
                    out=idx[:], in0=codesi[:],
                    scalar1=L - 1, scalar2=None,
                    op0=mybir.AluOpType.bitwise_and,
                )
                # tt = 1.0 where codes > L else 0.0 ; sign = 1 - 2*tt
                tt = spool.tile([P, NCODE], f32, tag="tt")
                nc.vector.tensor_scalar(
                    out=tt[:], in0=codes[:],
                    scalar1=float(L), scalar2=None,
                    op0=mybir.AluOpType.is_gt,
                )
                sg = spool.tile([P, NCODE], f32, tag="sg")
                nc.vector.tensor_scalar(
                    out=sg[:], in0=tt[:],
                    scalar1=-2.0, scalar2=1.0,
                    op0=mybir.AluOpType.mult, op1=mybir.AluOpType.add,
                )
                idxs.append(idx); tts.append(tt); sgs.append(sg)

            # Gather + permute + store stream.
            def emit_val(out_ap, in_ap, sg, tt, c):
                # val = sign*g + tt  (== 0.5 + sign*(g-0.5))
                nc.vector.tensor_scalar(
                    out=out_ap, in0=in_ap,
                    scalar1=sg[:, c : c + 1],
                    scalar2=tt[:, c : c + 1],
                    op0=mybir.AluOpType.mult,
                    op1=mybir.AluOpType.add,
                )

            for g in range(GROUPS):
                b0 = g * P
                idx, tt, sg = idxs[g], tts[g], sgs[g]
                od = opool.tile([P, 2 * 48 * 128], f32)
                od4 = od[:].rearrange("n (p k c) -> n p k c", p=2, k=48)
                for c in range(NCODE):
                    wide = c >= 14
                    gc = (gwpool if wide else gnpool).tile(
                        [P, ROW if wide else HROW], f16
                    )
                    gi = nc.gpsimd.indirect_dma_start(
                        out=gc[:],
                        out_offset=None,
                        in_=tn_t[:],
                        in_offset=bass.IndirectOffsetOnAxis(
                            ap=idx[:, c : c + 1], axis=0
                        ),
                        element_offset=HROW if 7 <= c < 14 else 0,
                    )
                    if c % 2:
                        gi.ins.queue = "qPoolDynamic1"
                    if wide:
                        col0 = (c - 7) * 8
                        glo = gc[:, 0:HROW].rearrange(
                            "n (p k c) -> n p k c", p=2, k=48
                        )
                        ghi = gc[:, HROW:ROW].rearrange(
                            "n (p k c) -> n p k c", p=2, k=48
                        )
                        emit_val(od4[:, :, :, col0 : col0 + 4], glo[:], sg, tt, c)
                        emit_val(od4[:, :, :, col0 + 4 : col0 + 8], ghi[:], sg, tt, c)
                    else:
                        col0 = c * 8 if c < 7 else (c - 7) * 8 + 4
                        gv = gc[:].rearrange("n (p k c) -> n p k c", p=2, k=48)
                        emit_val(od4[:, :, :, col0 : col0 + 4], gv[:], sg, tt, c)
                nc.sync.dma_start(
                    out=out3[b0 : b0 + P, :, D_LO:D_HI],
                    in_=od[:].rearrange("n (p f) -> n p f", p=2),
                )
            # Deferred fills: both rings co-drain the write tail after the
            # last store is pushed.
            if N_TAIL_FILLS:
                tail = regions[-N_TAIL_FILLS:]
                fill(nc.sync, tail[0::2])
                fill(nc.scalar, tail[1::2])
    nc.compile()
    return nc


def make_weights():
    w = np.tile((2.0 ** np.arange(NBITS)).astype(np.float32), NCODE)
    return np.broadcast_to(w, (P, NCODE * NBITS)).copy()


def make_tn(table):
    t = np.asarray(table).reshape(L, 2, 48, 8)
    tn = np.empty((L, ROW), dtype=np.float16)
    tn[:, :HROW] = t[:, :, :, 0:4].reshape(L, HROW)
    tn[:, HROW:] = t[:, :, :, 4:8].reshape(L, HROW)
    return tn


def make_in_maps(x, table):
    tn = make_tn(table)
    w = make_weights()
    return [
        {
            "x": np.ascontiguousarray(x[i * BC : (i + 1) * BC]),
            "table": tn,
            "w": w,
        }
        for i in range(NCORES)
    ]


_NC_CACHE = None


def _get_module():
    global _NC_CACHE
    if _NC_CACHE is None:
        _NC_CACHE = build_module()
    return _NC_CACHE


def kernel(x: np.ndarray, table: np.ndarray) -> np.ndarray:
    from concourse.bass_utils import run_bass_kernel_spmd

    x = np.asarray(x)
    table = np.asarray(table)
    assert x.shape == (BATCH, XCOLS) and table.shape == (L, 2, 48, 8)
    nc = _get_module()
    res = run_bass_kernel_spmd(nc, make_in_maps(x, table), core_ids=list(range(NCORES)))
    return np.concatenate([res.results[i]["out"] for i in range(NCORES)], axis=0)
